# revision 16
# baseline (speedup 1.0000x reference)
"""Multi-head causal attention on 8 TRN2 NeuronCores.

B=2, S=2048, D=1024, H=16 heads, head_dim=64. Tensor-parallel over heads:
core c owns heads {2c, 2c+1}. Each core:
  stage 1 (per 512-token piece): qT/kT/vT = W_c @ x.T (feature-major,
           bf16 matmuls, fp32 psum), then v -> token-major via PE
           transpose with a ones column per head appended (gives the
           softmax denominator for free).
  stage 2 (per batch, 512-wide q-piece, k-block pair): scoresT = k-major
           score blocks; the two heads' K=64 matmuls are emitted
           back-to-back so they run concurrently in different PE
           row-groups; exp on ScalarE (no max subtraction; |scores|/8
           is bounded ~4); causal masking via precomputed triangular
           masks; ctxT' accumulation = [v|1].T @ pT; normalize by the
           ones-row sum (reciprocal_approx_fast).
  stage 3: partial output projection outT_c = Wo_c.T-slice @ ctxT,
           emitted per token piece so it overlaps stage 2.
All psum pools coexist (2 + 4 + 2 banks) so stages pipeline end-to-end.
Host sums the 8 partial outputs and adds the bias.
"""
import numpy as np
import ml_dtypes

B, S, D, H = 2, 2048, 1024, 16
HD = 64          # head dim
NT = B * S       # 4096 tokens
P = 128          # partitions
NCORES = 8
HPC = 2          # heads per core

_cache = {}


def _build():
    import concourse.bass as bass
    import concourse.mybir as mybir
    from concourse import bacc
    import concourse.tile as tile
    from concourse.masks import make_identity

    BF16 = mybir.dt.bfloat16
    F32 = mybir.dt.float32
    Exp = mybir.ActivationFunctionType.Exp

    nc = bacc.Bacc("TRN2", target_bir_lowering=False, debug=False,
                   num_devices=NCORES)

    xT_d = nc.dram_tensor("xT", [D, NT], BF16, kind="ExternalInput")
    wq_d = nc.dram_tensor("wq", [P, D], BF16, kind="ExternalInput")
    wk_d = nc.dram_tensor("wk", [P, D], BF16, kind="ExternalInput")
    wv_d = nc.dram_tensor("wv", [P, D], BF16, kind="ExternalInput")
    wo_d = nc.dram_tensor("wo", [P, D], BF16, kind="ExternalInput")
    mask_d = nc.dram_tensor("mask", [P, 4 * 512], BF16, kind="ExternalInput")
    out_d = nc.dram_tensor("out", [D, NT], BF16, kind="ExternalOutput")

    NB = S // P          # 16 k-blocks per batch
    NM = S // 512        # 4 q-pieces per batch
    NP = NT // 512       # 8 token pieces overall
    VCB = 2 * (HD + 1)   # 130: v block cols: h0 feats+1, h1 feats+1

    with tile.TileContext(nc) as tc:
        with tc.tile_pool(name="const", bufs=1) as const, \
             tc.tile_pool(name="xp", bufs=1) as xp, \
             tc.tile_pool(name="qk", bufs=1) as qk, \
             tc.tile_pool(name="misc", bufs=8) as misc, \
             tc.tile_pool(name="stg", bufs=6) as stg, \
             tc.tile_pool(name="pt", bufs=10) as ptp, \
             tc.tile_pool(name="pp", bufs=2, space="PSUM") as pp, \
             tc.tile_pool(name="sc", bufs=2, space="PSUM") as scp, \
             tc.tile_pool(name="cx", bufs=2, space="PSUM") as cxp:

            # ---- constants / weights ----
            w_sb = {}
            for name, dd in (("wq", wq_d), ("wk", wk_d), ("wv", wv_d),
                             ("wo", wo_d)):
                t = const.tile([P, D], BF16, tag=name)
                nc.sync.dma_start(t[:], dd.ap())
                w_sb[name] = t
            mask_sb = const.tile([P, 4 * 512], BF16, tag="mask")
            nc.sync.dma_start(mask_sb[:], mask_d.ap())
            ident = const.tile([P, P], BF16, tag="ident")
            make_identity(nc, ident[:])

            # ---- stage 1: x loads + projections, per 512-token piece ----
            x_sb = [xp.tile([P, NT], BF16, tag=f"x{c}", name=f"x{c}")
                    for c in range(8)]
            qT = qk.tile([P, NT], BF16, tag="qT")
            kT = qk.tile([P, NT], BF16, tag="kT")
            vT = qk.tile([P, NT], BF16, tag="vT")
            v_sb = qk.tile([P, (NT // P) * VCB], BF16, tag="v")
            nc.gpsimd.memset(v_sb[:], 1.0)
            ctxT = qk.tile([P, NT], BF16, tag="ctxT")
            wo = w_sb["wo"]

            # hoist all x loads: the sync queue issues them before any
            # stage-3 out DMA can block it, and stage-1 matmuls gate on
            # the per-piece DMA semaphores as the data lands
            for n in range(NP):
                cols = slice(n * 512, (n + 1) * 512)
                for c in range(8):
                    nc.sync.dma_start(x_sb[c][:, cols],
                                      xT_d.ap()[c * P:(c + 1) * P, cols])

            def stage1_piece(n):
                cols = slice(n * 512, (n + 1) * 512)
                for wname, dst in (("wq", qT), ("wk", kT), ("wv", vT)):
                    w = w_sb[wname]
                    ps = pp.tile([P, 512], F32, tag="p1",
                                 name=f"p1_{wname}_{n}")
                    for cc in range(8):
                        nc.tensor.matmul(ps[:], w[:, cc * P:(cc + 1) * P],
                                         x_sb[cc][:, cols],
                                         start=(cc == 0), stop=(cc == 7))
                    if wname == "wv":
                        nc.vector.tensor_copy(dst[:, cols], ps[:])
                    else:
                        # ScalarE is idle early; offload q/k casts there
                        nc.scalar.copy(dst[:, cols], ps[:])
                # v -> token-major for the 4 blocks of this piece
                for t in range(4 * n, 4 * n + 4):
                    pst = pp.tile([P, P], BF16, tag="p1", name=f"ptr{t}")
                    nc.tensor.transpose(pst[:], vT[:, t * P:(t + 1) * P],
                                        ident[:])
                    # one 3D-AP copy places both heads' 64 feat cols
                    # (strides: head 65, feat 1), skipping the ones cols
                    dst3 = v_sb[:, t * VCB:(t + 1) * VCB].rearrange(
                        "p (h f) -> p h f", f=HD + 1)[:, :, 0:HD]
                    src3 = pst[:, :].rearrange("p (h f) -> p h f", f=HD)
                    nc.vector.tensor_copy(dst3, src3)

            def stage2_piece(b, m):
                qc0 = b * S + m * 512   # global q col of piece
                njs = 4 * m + 4
                ctx_ps = {hl: cxp.tile([HD + 1, 512], F32, tag="cx",
                                       name=f"cx_{b}_{m}_{hl}")
                          for hl in range(HPC)}
                for jg in range(0, njs, 2):
                    scs = {hl: scp.tile([P, 1024], F32, tag="sc",
                                        name=f"sc_{b}_{m}_{jg}_{hl}")
                           for hl in range(HPC)}
                    # interleave heads at the matmul level: adjacent MMs
                    # target different row groups -> run concurrently.
                    # tile_critical keeps the scheduler from separating
                    # the pairs.
                    with tc.tile_critical():
                        for t2 in range(2):
                            j = jg + t2
                            kc0 = b * S + j * P
                            for hl in range(HPC):
                                hbase = hl * HD
                                nc.tensor.matmul(
                                    scs[hl][:, t2 * 512:(t2 + 1) * 512],
                                    kT[hbase:hbase + HD, kc0:kc0 + P],
                                    qT[hbase:hbase + HD, qc0:qc0 + 512],
                                    start=True, stop=True,
                                    tile_position=(hbase, 0))
                    for hl in range(HPC):
                        pt = ptp.tile([P, 1024], BF16, tag="pt",
                                      name=f"pt_{b}_{m}_{jg}_{hl}")
                        nc.scalar.activation(pt[:], scs[hl][:], Exp,
                                             scale=0.125)
                        t = jg - 4 * m
                        if t >= 0:  # both js diagonal: one mask op
                            nc.vector.tensor_mul(
                                pt[:], pt[:],
                                mask_sb[:, t * 512:(t + 2) * 512])
                        for t2 in range(2):
                            j = jg + t2
                            vb = (b * NB + j) * VCB + hl * 65
                            nc.tensor.matmul(
                                ctx_ps[hl][:],
                                v_sb[:, vb:vb + HD + 1],
                                pt[:, t2 * 512:(t2 + 1) * 512],
                                start=(j == 0), stop=(j == njs - 1))
                # normalize: rows 0..63 ctx, row 64 = sum of exp
                for hl in range(HPC):
                    hbase = hl * HD
                    sm = misc.tile([1, 512], F32, tag="sm",
                                   name=f"sm_{b}_{m}_{hl}")
                    nc.vector.tensor_copy(sm[:], ctx_ps[hl][HD:HD + 1, :])
                    rc = misc.tile([1, 512], F32, tag="rc",
                                   name=f"rc_{b}_{m}_{hl}")
                    nc.vector.reciprocal_approx_fast(rc[:], sm[:])
                    bc = misc.tile([HD, 512], F32, tag="bc",
                                   name=f"bc_{b}_{m}_{hl}")
                    nc.gpsimd.partition_broadcast(bc[:], rc[:])
                    nc.vector.tensor_mul(
                        ctxT[hbase:hbase + HD, qc0:qc0 + 512],
                        ctx_ps[hl][0:HD, :], bc[:])

            def stage3_piece(n):
                cols = slice(n * 512, (n + 1) * 512)
                for f in range(D // P):
                    pso = pp.tile([P, 512], F32, tag="p1",
                                  name=f"p3_{f}_{n}")
                    nc.tensor.matmul(pso[:], wo[:, f * P:(f + 1) * P],
                                     ctxT[:, cols], start=True, stop=True)
                    st = stg.tile([P, 512], BF16, tag="st",
                                  name=f"st_{f}_{n}")
                    if f % 4 == 3:
                        nc.scalar.copy(st[:], pso[:])
                    else:
                        nc.vector.tensor_copy(st[:], pso[:])
                    nc.sync.dma_start(
                        out_d.ap()[f * P:(f + 1) * P, cols], st[:])

            # emission order: all of stage 1 piece-by-piece, then
            # attention; Tile's scheduler overlaps them via tile deps.
            # Interleave stage-1 and stage-2 pieces so the in-order PE
            # queue alternates projection work with attention work and
            # ScalarE is fed from the start. Each stage-2 piece (b, m)
            # needs stage-1 pieces b*4..b*4+m (k/v) and b*4+m (q).
            # b=1 runs m=(1,2,3,0) so the cheapest piece is the tail.
            # stage 3 lags one piece behind: its matmuls sit in the
            # in-order PE queue only after their normalize deps are
            # long done, so the PE never stalls at piece boundaries.
            prev = None
            plan = ["s1:0", "s1:1", "s2:0:0", "s1:2", "s2:0:1", "s1:3",
                    "s2:0:2", "s1:4", "s2:0:3", "s1:5", "s2:1:1", "s1:6",
                    "s2:1:2", "s1:7", "s2:1:3", "s2:1:0"]
            for step in plan:
                parts = step.split(":")
                if parts[0] == "s1":
                    stage1_piece(int(parts[1]))
                else:
                    b, m = int(parts[1]), int(parts[2])
                    stage2_piece(b, m)
                    if prev is not None:
                        stage3_piece(prev[0] * NM + prev[1])
                    prev = (b, m)
            stage3_piece(prev[0] * NM + prev[1])
    nc.compile()
    return nc


def _get_nc():
    if "nc" not in _cache:
        _cache["nc"] = _build()
    return _cache["nc"]


def _bf16(a):
    return np.ascontiguousarray(a).astype(ml_dtypes.bfloat16)


def _prepare_in_maps(x, Wq, Wk, Wv, Wo):
    xT = _bf16(np.asarray(x, np.float32).reshape(NT, D).T)
    mask = np.zeros((P, 4 * 512), np.float32)
    pp = np.arange(P)[:, None]
    for t in range(4):
        cc = np.arange(512)[None, :]
        mask[:, t * 512:(t + 1) * 512] = (pp <= cc - 128 * t)
    mask = _bf16(mask)

    def wlayout(Wslice):  # [128 feats, 1024 d] -> [p, cc*128+f]
        return _bf16(Wslice.reshape(P, 8, P).transpose(2, 1, 0)
                     .reshape(P, D))

    in_maps = []
    for c in range(NCORES):
        rows = slice(c * P, (c + 1) * P)
        in_maps.append({
            "xT": xT,
            "wq": wlayout(np.asarray(Wq, np.float32)[rows, :]),
            "wk": wlayout(np.asarray(Wk, np.float32)[rows, :]),
            "wv": wlayout(np.asarray(Wv, np.float32)[rows, :]),
            "wo": _bf16(np.asarray(Wo, np.float32)[:, rows].T),
            "mask": mask,
        })
    return in_maps


def _run(inputs, trace=False, tmpdir=None):
    from concourse.bass_utils import run_bass_kernel_spmd
    nc = _get_nc()
    in_maps = _prepare_in_maps(inputs["x"], inputs["Wq"], inputs["Wk"],
                               inputs["Wv"], inputs["Wo"])
    res = run_bass_kernel_spmd(nc, in_maps, core_ids=list(range(NCORES)),
                               trace=trace, tmpdir=tmpdir)
    acc = np.zeros((D, NT), np.float32)
    for r in res.results:
        acc += r["out"].astype(np.float32)
    out = acc.T.reshape(B, S, D) + np.asarray(inputs["bo"], np.float32)
    return out.astype(np.float32), res


def kernel(**inputs):
    out, _ = _run(inputs)
    return out


def kernel_traced(tmpdir=None, **inputs):
    out, res = _run(inputs, trace=True, tmpdir=tmpdir)
    return out, res


# revision 19
# speedup vs baseline: 1.3571x; 1.3571x over previous
"""Multi-head causal attention on 8 TRN2 NeuronCores.

B=2, S=2048, D=1024, H=16 heads, head_dim=64. Tensor-parallel over heads:
core c owns heads {2c, 2c+1}. Each core:
  stage 1 (per 512-token piece): qT/kT/vT = W_c @ x.T (feature-major,
           bf16 matmuls, fp32 psum), then v -> token-major via PE
           transpose with a ones column per head appended (gives the
           softmax denominator for free).
  stage 2 (per batch, 512-wide q-piece, k-block pair): scoresT = k-major
           score blocks; the two heads' K=64 matmuls are emitted
           back-to-back so they run concurrently in different PE
           row-groups; exp on ScalarE (no max subtraction; |scores|/8
           is bounded ~4); causal masking via precomputed triangular
           masks; ctxT' accumulation = [v|1].T @ pT; normalize by the
           ones-row sum (reciprocal_approx_fast).
  stage 3: partial output projection outT_c = Wo_c.T-slice @ ctxT,
           emitted per token piece so it overlaps stage 2.
All psum pools coexist (2 + 4 + 2 banks) so stages pipeline end-to-end.
Host sums the 8 partial outputs and adds the bias.
"""
import numpy as np
import ml_dtypes

B, S, D, H = 2, 2048, 1024, 16
HD = 64          # head dim
NT = B * S       # 4096 tokens
P = 128          # partitions
NCORES = 8
HPC = 2          # heads per core

_cache = {}


def _build():
    import concourse.bass as bass
    import concourse.mybir as mybir
    from concourse import bacc
    import concourse.tile as tile
    from concourse.masks import make_identity

    BF16 = mybir.dt.bfloat16
    F32 = mybir.dt.float32
    Exp = mybir.ActivationFunctionType.Exp

    nc = bacc.Bacc("TRN2", target_bir_lowering=False, debug=False,
                   num_devices=NCORES)

    xT_d = nc.dram_tensor("xT", [D, NT], BF16, kind="ExternalInput")
    wq_d = nc.dram_tensor("wq", [P, D], BF16, kind="ExternalInput")
    wk_d = nc.dram_tensor("wk", [P, D], BF16, kind="ExternalInput")
    wv_d = nc.dram_tensor("wv", [P, D], BF16, kind="ExternalInput")
    wo_d = nc.dram_tensor("wo", [P, D], BF16, kind="ExternalInput")
    mask_d = nc.dram_tensor("mask", [P, 4 * 512], BF16, kind="ExternalInput")
    out_d = nc.dram_tensor("out", [D, NT], BF16, kind="ExternalOutput")

    NB = S // P          # 16 k-blocks per batch
    NM = S // 512        # 4 q-pieces per batch
    NP = NT // 512       # 8 token pieces overall
    VCB = 2 * (HD + 1)   # 130: v block cols: h0 feats+1, h1 feats+1

    with tile.TileContext(nc) as tc:
        with tc.tile_pool(name="const", bufs=1) as const, \
             tc.tile_pool(name="xp", bufs=1) as xp, \
             tc.tile_pool(name="qk", bufs=1) as qk, \
             tc.tile_pool(name="misc", bufs=8) as misc, \
             tc.tile_pool(name="stg", bufs=6) as stg, \
             tc.tile_pool(name="pt", bufs=10) as ptp, \
             tc.tile_pool(name="pp", bufs=2, space="PSUM") as pp, \
             tc.tile_pool(name="sc", bufs=2, space="PSUM") as scp, \
             tc.tile_pool(name="cx", bufs=2, space="PSUM") as cxp:

            # ---- constants / weights ----
            w_sb = {}
            for name, dd in (("wq", wq_d), ("wk", wk_d), ("wv", wv_d),
                             ("wo", wo_d)):
                t = const.tile([P, D], BF16, tag=name)
                nc.sync.dma_start(t[:], dd.ap())
                w_sb[name] = t
            mask_sb = const.tile([P, 4 * 512], BF16, tag="mask")
            nc.sync.dma_start(mask_sb[:], mask_d.ap())
            ident = const.tile([P, P], BF16, tag="ident")
            make_identity(nc, ident[:])

            # ---- stage 1: x loads + projections, per 512-token piece ----
            x_sb = [xp.tile([P, NT], BF16, tag=f"x{c}", name=f"x{c}")
                    for c in range(8)]
            qT = qk.tile([P, NT], BF16, tag="qT")
            kT = qk.tile([P, NT], BF16, tag="kT")
            vT = qk.tile([P, NT], BF16, tag="vT")
            v_sb = qk.tile([P, (NT // P) * VCB], BF16, tag="v")
            nc.gpsimd.memset(v_sb[:], 1.0)
            ctxT = qk.tile([P, NT], BF16, tag="ctxT")
            wo = w_sb["wo"]

            # hoist all x loads: the sync queue issues them before any
            # stage-3 out DMA can block it, and stage-1 matmuls gate on
            # the per-piece DMA semaphores as the data lands
            for n in range(NP):
                cols = slice(n * 512, (n + 1) * 512)
                for c in range(8):
                    nc.sync.dma_start(x_sb[c][:, cols],
                                      xT_d.ap()[c * P:(c + 1) * P, cols])

            def s1_proj(n, wname, dst):
                cols = slice(n * 512, (n + 1) * 512)
                w = w_sb[wname]
                ps = pp.tile([P, 512], F32, tag="p1",
                             name=f"p1_{wname}_{n}")
                for cc in range(8):
                    nc.tensor.matmul(ps[:], w[:, cc * P:(cc + 1) * P],
                                     x_sb[cc][:, cols],
                                     start=(cc == 0), stop=(cc == 7))
                if wname == "wv":
                    nc.vector.tensor_copy(dst[:, cols], ps[:])
                else:
                    # ScalarE is idle early; offload q/k casts there
                    nc.scalar.copy(dst[:, cols], ps[:])

            def s1_vtrans(n):
                # v -> token-major for the 4 blocks of this piece
                for t in range(4 * n, 4 * n + 4):
                    pst = pp.tile([P, P], BF16, tag="p1", name=f"ptr{t}")
                    nc.tensor.transpose(pst[:], vT[:, t * P:(t + 1) * P],
                                        ident[:])
                    # one 3D-AP copy places both heads' 64 feat cols
                    # (strides: head 65, feat 1), skipping the ones cols
                    dst3 = v_sb[:, t * VCB:(t + 1) * VCB].rearrange(
                        "p (h f) -> p h f", f=HD + 1)[:, :, 0:HD]
                    src3 = pst[:, :].rearrange("p (h f) -> p h f", f=HD)
                    nc.vector.tensor_copy(dst3, src3)

            def s2_group(b, m, jg, ctx_ps):
                qc0 = b * S + m * 512
                njs = 4 * m + 4
                scs = {hl: scp.tile([P, 1024], F32, tag="sc",
                                    name=f"sc_{b}_{m}_{jg}_{hl}")
                       for hl in range(HPC)}
                # interleave heads at the matmul level: adjacent MMs
                # target different row groups -> run concurrently
                for t2 in range(2):
                    j = jg + t2
                    kc0 = b * S + j * P
                    for hl in range(HPC):
                        hbase = hl * HD
                        nc.tensor.matmul(
                            scs[hl][:, t2 * 512:(t2 + 1) * 512],
                            kT[hbase:hbase + HD, kc0:kc0 + P],
                            qT[hbase:hbase + HD, qc0:qc0 + 512],
                            start=True, stop=True,
                            tile_position=(hbase, 0))
                for hl in range(HPC):
                    pt = ptp.tile([P, 1024], BF16, tag="pt",
                                  name=f"pt_{b}_{m}_{jg}_{hl}")
                    nc.scalar.activation(pt[:], scs[hl][:], Exp,
                                         scale=0.125)
                    t = jg - 4 * m
                    if t >= 0:  # both js diagonal: one mask op
                        nc.vector.tensor_mul(
                            pt[:], pt[:],
                            mask_sb[:, t * 512:(t + 2) * 512])
                    for t2 in range(2):
                        j = jg + t2
                        vb = (b * NB + j) * VCB + hl * 65
                        nc.tensor.matmul(
                            ctx_ps[hl][:],
                            v_sb[:, vb:vb + HD + 1],
                            pt[:, t2 * 512:(t2 + 1) * 512],
                            start=(j == 0), stop=(j == njs - 1))

            def s2_normalize(b, m, ctx_ps):
                # normalize: rows 0..63 ctx, row 64 = sum of exp
                qc0 = b * S + m * 512
                for hl in range(HPC):
                    hbase = hl * HD
                    sm = misc.tile([1, 512], F32, tag="sm",
                                   name=f"sm_{b}_{m}_{hl}")
                    nc.vector.tensor_copy(sm[:], ctx_ps[hl][HD:HD + 1, :])
                    rc = misc.tile([1, 512], F32, tag="rc",
                                   name=f"rc_{b}_{m}_{hl}")
                    nc.vector.reciprocal_approx_fast(rc[:], sm[:])
                    bc = misc.tile([HD, 512], F32, tag="bc",
                                   name=f"bc_{b}_{m}_{hl}")
                    nc.gpsimd.partition_broadcast(bc[:], rc[:])
                    nc.vector.tensor_mul(
                        ctxT[hbase:hbase + HD, qc0:qc0 + 512],
                        ctx_ps[hl][0:HD, :], bc[:])

            def s3_half(n, half):
                cols = slice(n * 512, (n + 1) * 512)
                for f in range(half * 4, half * 4 + 4):
                    pso = pp.tile([P, 512], F32, tag="p1",
                                  name=f"p3_{f}_{n}")
                    nc.tensor.matmul(pso[:], wo[:, f * P:(f + 1) * P],
                                     ctxT[:, cols], start=True, stop=True)
                    st = stg.tile([P, 512], BF16, tag="st",
                                  name=f"st_{f}_{n}")
                    if f % 4 == 3:
                        nc.scalar.copy(st[:], pso[:])
                    else:
                        nc.vector.tensor_copy(st[:], pso[:])
                    nc.sync.dma_start(
                        out_d.ap()[f * P:(f + 1) * P, cols], st[:])

            # ---- emission: zipper stage-1 sub-units between stage-2
            # groups so the in-order PE queue never has a long run of
            # projection work starving ScalarE of score matmuls, nor
            # vice versa. Q1 = stage-1 sub-units (~2us PE each); Q2 =
            # attention groups / normalize / stage-3 halves. A stage-2
            # group (b,m,jg) needs stage-1 pieces <= b*4+max(m,(jg+1)//4)
            # complete; the zipper emits Q1 eagerly when Q2 is blocked,
            # else ~1 Q1 unit per 2 Q2 units until Q1 drains.
            q1 = []
            for n in range(NP):
                q1 += [(n, lambda n=n: s1_proj(n, "wq", qT)),
                       (n, lambda n=n: s1_proj(n, "wk", kT)),
                       (n, lambda n=n: s1_proj(n, "wv", vT)),
                       (n, lambda n=n: s1_vtrans(n))]
            q2 = []  # (req_piece, fn)
            ctx_tiles = {}

            def mk_ctx(b, m):
                ctx_tiles[(b, m)] = {
                    hl: cxp.tile([HD + 1, 512], F32, tag="cx",
                                 name=f"cx_{b}_{m}_{hl}")
                    for hl in range(HPC)}

            order = [(0, 0), (0, 1), (0, 2), (0, 3),
                     (1, 1), (1, 2), (1, 3), (1, 0)]
            prev = None
            for b, m in order:
                q2.append((b * NM + m,
                           lambda b=b, m=m: mk_ctx(b, m)))
                for jg in range(0, 4 * m + 4, 2):
                    req = b * NM + max(m, (jg + 1) // 4)
                    q2.append((req, lambda b=b, m=m, jg=jg:
                               s2_group(b, m, jg, ctx_tiles[(b, m)])))
                q2.append((b * NM + m, lambda b=b, m=m:
                           s2_normalize(b, m, ctx_tiles[(b, m)])))
                if prev is not None:
                    pn = prev[0] * NM + prev[1]
                    q2.append((pn, lambda pn=pn: s3_half(pn, 0)))
                    q2.append((pn, lambda pn=pn: s3_half(pn, 1)))
                prev = (b, m)
            pn = prev[0] * NM + prev[1]
            q2.append((pn, lambda pn=pn: s3_half(pn, 0)))
            q2.append((pn, lambda pn=pn: s3_half(pn, 1)))

            i1 = i2 = 0
            done1 = -1  # highest fully-emitted stage-1 piece
            credit = 0
            while i2 < len(q2) or i1 < len(q1):
                can2 = i2 < len(q2) and q2[i2][0] <= done1
                if i1 < len(q1) and (not can2 or credit <= 0):
                    n, fn = q1[i1]
                    fn()
                    if i1 + 1 >= len(q1) or q1[i1 + 1][0] != n:
                        done1 = n
                    i1 += 1
                    credit += 2
                else:
                    q2[i2][1]()
                    i2 += 1
                    credit -= 1
    nc.compile()
    return nc


def _get_nc():
    if "nc" not in _cache:
        _cache["nc"] = _build()
    return _cache["nc"]


def _bf16(a):
    return np.ascontiguousarray(a).astype(ml_dtypes.bfloat16)


def _prepare_in_maps(x, Wq, Wk, Wv, Wo):
    xT = _bf16(np.asarray(x, np.float32).reshape(NT, D).T)
    mask = np.zeros((P, 4 * 512), np.float32)
    pp = np.arange(P)[:, None]
    for t in range(4):
        cc = np.arange(512)[None, :]
        mask[:, t * 512:(t + 1) * 512] = (pp <= cc - 128 * t)
    mask = _bf16(mask)

    def wlayout(Wslice):  # [128 feats, 1024 d] -> [p, cc*128+f]
        return _bf16(Wslice.reshape(P, 8, P).transpose(2, 1, 0)
                     .reshape(P, D))

    in_maps = []
    for c in range(NCORES):
        rows = slice(c * P, (c + 1) * P)
        in_maps.append({
            "xT": xT,
            "wq": wlayout(np.asarray(Wq, np.float32)[rows, :]),
            "wk": wlayout(np.asarray(Wk, np.float32)[rows, :]),
            "wv": wlayout(np.asarray(Wv, np.float32)[rows, :]),
            "wo": _bf16(np.asarray(Wo, np.float32)[:, rows].T),
            "mask": mask,
        })
    return in_maps


def _run(inputs, trace=False, tmpdir=None):
    from concourse.bass_utils import run_bass_kernel_spmd
    nc = _get_nc()
    in_maps = _prepare_in_maps(inputs["x"], inputs["Wq"], inputs["Wk"],
                               inputs["Wv"], inputs["Wo"])
    res = run_bass_kernel_spmd(nc, in_maps, core_ids=list(range(NCORES)),
                               trace=trace, tmpdir=tmpdir)
    acc = np.zeros((D, NT), np.float32)
    for r in res.results:
        acc += r["out"].astype(np.float32)
    out = acc.T.reshape(B, S, D) + np.asarray(inputs["bo"], np.float32)
    return out.astype(np.float32), res


def kernel(**inputs):
    out, _ = _run(inputs)
    return out


def kernel_traced(tmpdir=None, **inputs):
    out, res = _run(inputs, trace=True, tmpdir=tmpdir)
    return out, res


# revision 20
# speedup vs baseline: 1.4556x; 1.0726x over previous
"""Multi-head causal attention on 8 TRN2 NeuronCores.

B=2, S=2048, D=1024, H=16 heads, head_dim=64. Tensor-parallel over heads:
core c owns heads {2c, 2c+1}. Each core:
  stage 1 (per 512-token piece): qT/kT/vT = W_c @ x.T (feature-major,
           bf16 matmuls, fp32 psum), then v -> token-major via PE
           transpose with a ones column per head appended (gives the
           softmax denominator for free).
  stage 2 (per batch, 512-wide q-piece, k-block pair): scoresT = k-major
           score blocks; the two heads' K=64 matmuls are emitted
           back-to-back so they run concurrently in different PE
           row-groups; exp on ScalarE (no max subtraction; |scores|/8
           is bounded ~4); causal masking via precomputed triangular
           masks; ctxT' accumulation = [v|1].T @ pT; normalize by the
           ones-row sum (reciprocal_approx_fast).
  stage 3: partial output projection outT_c = Wo_c.T-slice @ ctxT,
           emitted per token piece so it overlaps stage 2.
All psum pools coexist (2 + 4 + 2 banks) so stages pipeline end-to-end.
Host sums the 8 partial outputs and adds the bias.
"""
import numpy as np
import ml_dtypes

B, S, D, H = 2, 2048, 1024, 16
HD = 64          # head dim
NT = B * S       # 4096 tokens
P = 128          # partitions
NCORES = 8
HPC = 2          # heads per core

_cache = {}


def _build():
    import concourse.bass as bass
    import concourse.mybir as mybir
    from concourse import bacc
    import concourse.tile as tile
    from concourse.masks import make_identity

    BF16 = mybir.dt.bfloat16
    F32 = mybir.dt.float32
    Exp = mybir.ActivationFunctionType.Exp

    nc = bacc.Bacc("TRN2", target_bir_lowering=False, debug=False,
                   num_devices=NCORES)

    xT_d = nc.dram_tensor("xT", [D, NT], BF16, kind="ExternalInput")
    wq_d = nc.dram_tensor("wq", [P, D], BF16, kind="ExternalInput")
    wk_d = nc.dram_tensor("wk", [P, D], BF16, kind="ExternalInput")
    wv_d = nc.dram_tensor("wv", [P, D], BF16, kind="ExternalInput")
    wo_d = nc.dram_tensor("wo", [P, D], BF16, kind="ExternalInput")
    mask_d = nc.dram_tensor("mask", [P, 4 * 512], BF16, kind="ExternalInput")
    out_d = nc.dram_tensor("out", [D, NT], BF16, kind="ExternalOutput")

    NB = S // P          # 16 k-blocks per batch
    NM = S // 512        # 4 q-pieces per batch
    NP = NT // 512       # 8 token pieces overall
    VCB = 2 * (HD + 1)   # 130: v block cols: h0 feats+1, h1 feats+1

    with tile.TileContext(nc) as tc:
        with tc.tile_pool(name="const", bufs=1) as const, \
             tc.tile_pool(name="xp", bufs=1) as xp, \
             tc.tile_pool(name="qk", bufs=1) as qk, \
             tc.tile_pool(name="misc", bufs=8) as misc, \
             tc.tile_pool(name="stg", bufs=6) as stg, \
             tc.tile_pool(name="pt", bufs=10) as ptp, \
             tc.tile_pool(name="pp", bufs=2, space="PSUM") as pp, \
             tc.tile_pool(name="sc", bufs=2, space="PSUM") as scp, \
             tc.tile_pool(name="cx", bufs=2, space="PSUM") as cxp:

            # ---- constants / weights ----
            w_sb = {}
            for name, dd in (("wq", wq_d), ("wk", wk_d), ("wv", wv_d),
                             ("wo", wo_d)):
                t = const.tile([P, D], BF16, tag=name)
                nc.sync.dma_start(t[:], dd.ap())
                w_sb[name] = t
            mask_sb = const.tile([P, 4 * 512], BF16, tag="mask")
            nc.sync.dma_start(mask_sb[:], mask_d.ap())
            ident = const.tile([P, P], BF16, tag="ident")
            make_identity(nc, ident[:])

            # ---- stage 1: x loads + projections, per 512-token piece ----
            x_sb = [xp.tile([P, NT], BF16, tag=f"x{c}", name=f"x{c}")
                    for c in range(8)]
            qT = qk.tile([P, NT], BF16, tag="qT")
            kT = qk.tile([P, NT], BF16, tag="kT")
            vT = qk.tile([P, NT], BF16, tag="vT")
            v_sb = qk.tile([P, (NT // P) * VCB], BF16, tag="v")
            nc.gpsimd.memset(v_sb[:], 1.0)
            ctxT = qk.tile([P, NT], BF16, tag="ctxT")
            wo = w_sb["wo"]

            # hoist all x loads: the sync queue issues them before any
            # stage-3 out DMA can block it, and stage-1 matmuls gate on
            # the per-piece DMA semaphores as the data lands
            for n in range(NP):
                cols = slice(n * 512, (n + 1) * 512)
                for c in range(8):
                    nc.sync.dma_start(x_sb[c][:, cols],
                                      xT_d.ap()[c * P:(c + 1) * P, cols])

            def s1_proj(n, wname, dst):
                cols = slice(n * 512, (n + 1) * 512)
                w = w_sb[wname]
                ps = pp.tile([P, 512], F32, tag="p1",
                             name=f"p1_{wname}_{n}")
                for cc in range(8):
                    nc.tensor.matmul(ps[:], w[:, cc * P:(cc + 1) * P],
                                     x_sb[cc][:, cols],
                                     start=(cc == 0), stop=(cc == 7))
                if wname == "wv":
                    nc.vector.tensor_copy(dst[:, cols], ps[:])
                else:
                    # ScalarE is idle early; offload q/k casts there
                    nc.scalar.copy(dst[:, cols], ps[:])

            def s1_vtrans(n):
                # v -> token-major for the 4 blocks of this piece
                for t in range(4 * n, 4 * n + 4):
                    pst = pp.tile([P, P], BF16, tag="p1", name=f"ptr{t}")
                    nc.tensor.transpose(pst[:], vT[:, t * P:(t + 1) * P],
                                        ident[:])
                    # one 3D-AP copy places both heads' 64 feat cols
                    # (strides: head 65, feat 1), skipping the ones cols
                    dst3 = v_sb[:, t * VCB:(t + 1) * VCB].rearrange(
                        "p (h f) -> p h f", f=HD + 1)[:, :, 0:HD]
                    src3 = pst[:, :].rearrange("p (h f) -> p h f", f=HD)
                    nc.vector.tensor_copy(dst3, src3)

            def s2_scores(b, m, jg):
                qc0 = b * S + m * 512
                scs = {hl: scp.tile([P, 1024], F32, tag="sc",
                                    name=f"sc_{b}_{m}_{jg}_{hl}")
                       for hl in range(HPC)}
                # interleave heads at the matmul level: adjacent MMs
                # target different row groups -> run concurrently
                for t2 in range(2):
                    j = jg + t2
                    kc0 = b * S + j * P
                    for hl in range(HPC):
                        hbase = hl * HD
                        nc.tensor.matmul(
                            scs[hl][:, t2 * 512:(t2 + 1) * 512],
                            kT[hbase:hbase + HD, kc0:kc0 + P],
                            qT[hbase:hbase + HD, qc0:qc0 + 512],
                            start=True, stop=True,
                            tile_position=(hbase, 0))
                return scs

            def s2_consume(b, m, jg, scs, ctx_ps):
                njs = 4 * m + 4
                for hl in range(HPC):
                    pt = ptp.tile([P, 1024], BF16, tag="pt",
                                  name=f"pt_{b}_{m}_{jg}_{hl}")
                    nc.scalar.activation(pt[:], scs[hl][:], Exp,
                                         scale=0.125)
                    t = jg - 4 * m
                    if t >= 0:  # both js diagonal: one mask op
                        nc.vector.tensor_mul(
                            pt[:], pt[:],
                            mask_sb[:, t * 512:(t + 2) * 512])
                    for t2 in range(2):
                        j = jg + t2
                        vb = (b * NB + j) * VCB + hl * 65
                        nc.tensor.matmul(
                            ctx_ps[hl][:],
                            v_sb[:, vb:vb + HD + 1],
                            pt[:, t2 * 512:(t2 + 1) * 512],
                            start=(j == 0), stop=(j == njs - 1))

            def s2_normalize(b, m, ctx_ps):
                # normalize: rows 0..63 ctx, row 64 = sum of exp
                qc0 = b * S + m * 512
                for hl in range(HPC):
                    hbase = hl * HD
                    sm = misc.tile([1, 512], F32, tag="sm",
                                   name=f"sm_{b}_{m}_{hl}")
                    nc.vector.tensor_copy(sm[:], ctx_ps[hl][HD:HD + 1, :])
                    rc = misc.tile([1, 512], F32, tag="rc",
                                   name=f"rc_{b}_{m}_{hl}")
                    nc.vector.reciprocal_approx_fast(rc[:], sm[:])
                    bc = misc.tile([HD, 512], F32, tag="bc",
                                   name=f"bc_{b}_{m}_{hl}")
                    nc.gpsimd.partition_broadcast(bc[:], rc[:])
                    nc.vector.tensor_mul(
                        ctxT[hbase:hbase + HD, qc0:qc0 + 512],
                        ctx_ps[hl][0:HD, :], bc[:])

            def s3_half(n, half):
                cols = slice(n * 512, (n + 1) * 512)
                for f in range(half * 4, half * 4 + 4):
                    pso = pp.tile([P, 512], F32, tag="p1",
                                  name=f"p3_{f}_{n}")
                    nc.tensor.matmul(pso[:], wo[:, f * P:(f + 1) * P],
                                     ctxT[:, cols], start=True, stop=True)
                    st = stg.tile([P, 512], BF16, tag="st",
                                  name=f"st_{f}_{n}")
                    if f % 4 == 3:
                        nc.scalar.copy(st[:], pso[:])
                    else:
                        nc.vector.tensor_copy(st[:], pso[:])
                    nc.sync.dma_start(
                        out_d.ap()[f * P:(f + 1) * P, cols], st[:])

            # ---- emission: zipper stage-1 sub-units between stage-2
            # groups so the in-order PE queue never has a long run of
            # projection work starving ScalarE of score matmuls, nor
            # vice versa. Q1 = stage-1 sub-units (~2us PE each); Q2 =
            # attention groups / normalize / stage-3 halves. A stage-2
            # group (b,m,jg) needs stage-1 pieces <= b*4+max(m,(jg+1)//4)
            # complete; the zipper emits Q1 eagerly when Q2 is blocked,
            # else ~1 Q1 unit per 2 Q2 units until Q1 drains.
            q1 = []
            for n in range(NP):
                q1 += [(n, lambda n=n: s1_proj(n, "wq", qT)),
                       (n, lambda n=n: s1_proj(n, "wk", kT)),
                       (n, lambda n=n: s1_proj(n, "wv", vT)),
                       (n, lambda n=n: s1_vtrans(n))]
            ctx_tiles = {}

            def mk_ctx(b, m):
                ctx_tiles[(b, m)] = {
                    hl: cxp.tile([HD + 1, 512], F32, tag="cx",
                                 name=f"cx_{b}_{m}_{hl}")
                    for hl in range(HPC)}

            order = [(0, 0), (0, 1), (0, 2), (0, 3),
                     (1, 1), (1, 2), (1, 3), (1, 0)]
            # score matmuls skewed one group ahead of their exp/ctx
            # consumers in program order, so the PE prefers feeding
            # ScalarE over draining ctx work
            sc_tiles = {}
            pend = []  # (b, m, jg) emitted scores awaiting consume

            def emit_scores(b, m, jg):
                sc_tiles[(b, m, jg)] = s2_scores(b, m, jg)
                pend.append((b, m, jg))

            def emit_consume():
                bb, mm, jj = pend.pop(0)
                s2_consume(bb, mm, jj, sc_tiles.pop((bb, mm, jj)),
                           ctx_tiles[(bb, mm)])

            steps = []  # (req_piece, kind, args)
            for b, m in order:
                steps.append((b * NM + m, "ctx", (b, m)))
                for jg in range(0, 4 * m + 4, 2):
                    req = b * NM + max(m, (jg + 1) // 4)
                    steps.append((req, "grp", (b, m, jg)))
                steps.append((b * NM + m, "norm", (b, m)))
                steps.append((b * NM + m, "s3", (b, m)))

            q2 = []
            prevgrp = None
            s3_pending = []
            for req, kind, args in steps:
                if kind == "ctx":
                    q2.append((req, lambda a=args: mk_ctx(*a)))
                elif kind == "grp":
                    q2.append((req, lambda a=args: emit_scores(*a)))
                    if prevgrp is not None:
                        q2.append((prevgrp[0], lambda: emit_consume()))
                    prevgrp = (req, args)
                elif kind == "norm":
                    if prevgrp is not None:
                        q2.append((prevgrp[0], lambda: emit_consume()))
                        prevgrp = None
                    q2.append((req, lambda a=args: s2_normalize(
                        a[0], a[1], ctx_tiles[(a[0], a[1])])))
                    # lagged stage-3 of the previous piece
                    if s3_pending:
                        pn = s3_pending.pop(0)
                        q2.append((pn, lambda pn=pn: s3_half(pn, 0)))
                        q2.append((pn, lambda pn=pn: s3_half(pn, 1)))
                elif kind == "s3":
                    s3_pending.append(args[0] * NM + args[1])
            for pn in s3_pending:
                q2.append((pn, lambda pn=pn: s3_half(pn, 0)))
                q2.append((pn, lambda pn=pn: s3_half(pn, 1)))

            i1 = i2 = 0
            done1 = -1  # highest fully-emitted stage-1 piece
            credit = 0
            while i2 < len(q2) or i1 < len(q1):
                can2 = i2 < len(q2) and q2[i2][0] <= done1
                if i1 < len(q1) and (not can2 or credit <= 0):
                    n, fn = q1[i1]
                    fn()
                    if i1 + 1 >= len(q1) or q1[i1 + 1][0] != n:
                        done1 = n
                    i1 += 1
                    credit += 2
                else:
                    q2[i2][1]()
                    i2 += 1
                    credit -= 1
    nc.compile()
    return nc


def _get_nc():
    if "nc" not in _cache:
        _cache["nc"] = _build()
    return _cache["nc"]


def _bf16(a):
    return np.ascontiguousarray(a).astype(ml_dtypes.bfloat16)


def _prepare_in_maps(x, Wq, Wk, Wv, Wo):
    xT = _bf16(np.asarray(x, np.float32).reshape(NT, D).T)
    mask = np.zeros((P, 4 * 512), np.float32)
    pp = np.arange(P)[:, None]
    for t in range(4):
        cc = np.arange(512)[None, :]
        mask[:, t * 512:(t + 1) * 512] = (pp <= cc - 128 * t)
    mask = _bf16(mask)

    def wlayout(Wslice):  # [128 feats, 1024 d] -> [p, cc*128+f]
        return _bf16(Wslice.reshape(P, 8, P).transpose(2, 1, 0)
                     .reshape(P, D))

    in_maps = []
    for c in range(NCORES):
        rows = slice(c * P, (c + 1) * P)
        in_maps.append({
            "xT": xT,
            "wq": wlayout(np.asarray(Wq, np.float32)[rows, :]),
            "wk": wlayout(np.asarray(Wk, np.float32)[rows, :]),
            "wv": wlayout(np.asarray(Wv, np.float32)[rows, :]),
            "wo": _bf16(np.asarray(Wo, np.float32)[:, rows].T),
            "mask": mask,
        })
    return in_maps


def _run(inputs, trace=False, tmpdir=None):
    from concourse.bass_utils import run_bass_kernel_spmd
    nc = _get_nc()
    in_maps = _prepare_in_maps(inputs["x"], inputs["Wq"], inputs["Wk"],
                               inputs["Wv"], inputs["Wo"])
    res = run_bass_kernel_spmd(nc, in_maps, core_ids=list(range(NCORES)),
                               trace=trace, tmpdir=tmpdir)
    acc = np.zeros((D, NT), np.float32)
    for r in res.results:
        acc += r["out"].astype(np.float32)
    out = acc.T.reshape(B, S, D) + np.asarray(inputs["bo"], np.float32)
    return out.astype(np.float32), res


def kernel(**inputs):
    out, _ = _run(inputs)
    return out


def kernel_traced(tmpdir=None, **inputs):
    out, res = _run(inputs, trace=True, tmpdir=tmpdir)
    return out, res
